# revision 23
# baseline (speedup 1.0000x reference)
"""Trainium2 Bass kernel for nn_ColRepeatCausalLinear.

Math: reference computes out = x @ W + bias with
    W[s, t] = v[t] * d^(t-s)  for t >= s, else 0,   d = clip(decay_value, 0.9, 1)
which factorizes as a decayed prefix scan along S:
    y[b, e, t] = d * y[b, e, t-1] + x[b, e, t]
    out[b, e, t] = v[t] * y[b, e, t] + bias[t]
i.e. O(B*E*S) work instead of the O(B*E*S^2) dense matmul.

Mapping: data-parallel over B across 8 NeuronCores (x[b] per core, params
replicated). Per core the kernel is DMA-bound (the scan+scale is one fused
Vector-engine op per 128x2048 tile), so I/O is done in fp16: the host casts
x/v to fp16 (quantization ~5e-4 L2 rel err, budget is 2e-2), the device
scans in fp32 internally (DVE ports upconvert), and the fp16 result is
upcast on the host. That halves HBM traffic: 8.4 MB/core instead of 16.8.
All 16+1 tiles live in SBUF simultaneously (68 KiB/partition of 208), so
every load issues at t=0 with no buffer-reuse (WAR) stalls, spread over the
three DGE rings (SP + ACT HWDGE, GpSimd SWDGE).

Hardcoded problem shapes: x (8, 1024, 2048) f32, weight (1, 2048),
bias (2048,), decay_value (1,).
"""

import numpy as np

import concourse.bacc as bacc
import concourse.mybir as mybir
from concourse.tile import TileContext
from concourse.bass_utils import run_bass_kernel_spmd

B, E, S = 8, 1024, 2048
P = 128
N_CORES = 8
F32 = mybir.dt.float32
F16 = mybir.dt.float16
I8 = mybir.dt.int8

_cache = {}

# Fused custom DVE op: out[p,k] = (sum_{j<=k} x[p,j]) * v[p,k] — the whole
# d=1 kernel body in ONE Vector-engine instruction (the stock path needs a
# 2-cyc/elem TensorTensorScan plus a 1-cyc/elem tensor_mul). Registered at
# runtime into dve_ops.OPS; sha self-pinned since this op isn't in-tree.
_FUSED_OP = None
try:
    from concourse import dve_ops as _dops
    from concourse.dve_spec import AluOp as _AluOp, Spec as _Spec
    from concourse.dve_spec import Src0 as _Src0, Src1 as _Src1, scan as _scan
    from concourse.dve_spec import lower as _lower
    from concourse.dve_uop import DveOpSpec as _DveOpSpec

    _FUSED_NAME = "CUMSUM_VSCALE_ANT"
    if _FUSED_NAME in _dops._SUB_OPCODE_FOR_NAME:
        _FUSED_OP = next(o for o in _dops.OPS if o.name == _FUSED_NAME)
    else:
        _fspec = _Spec(body=_scan(_AluOp.ADD, _Src0) * _Src1)
        _row = _dops._CUSTOM_DVE_ROW_BASE + len(_dops.OPS)
        assert _row < 0x20
        _dops._SUB_OPCODE_FOR_NAME[_FUSED_NAME] = _row
        _sha = {}
        for _ver in ("v3", "v4"):
            try:
                _sha[_ver] = _DveOpSpec(
                    name=_FUSED_NAME,
                    opcode=_row,
                    uops=_lower(_fspec, ver=_ver),
                    rd1_en=_dops.has_src1(_fspec),
                ).sha(_ver)
            except Exception:
                pass
        _FUSED_OP = _dops.DveOp(_FUSED_NAME, _fspec, subdim=False, uops_sha=_sha)
        _dops.OPS.append(_FUSED_OP)
        _dops.CUSTOM_DVE_SPECS[_FUSED_NAME] = _fspec
except Exception:
    _FUSED_OP = None


def _build_fp16():
    """Fast path: d == 1, no bias, fp16 I/O, fused scan*v DVE op.

    Raw bass (no TileContext): the dependency structure is static and
    tiny (17 DMAs, 8 scans), so explicit semaphores cost nothing and the
    TileContext exit epilogue (double all-engine barrier + event-sem
    RANGE_CLEAR + per-ring drains, ~8 us of a ~38 us kernel) collapses
    to one drain + sem-only barrier from Block exit.

    Schedule: loads split across the SP and ACT HWDGE rings in tile
    order (vb + tile0 split first so scan0 starts ~3 us in); the DVE
    chain of 8 fused scan*v ops is the latency backbone; stores issue
    from each ring as scans complete, queued after all loads so a
    blocked store never delays a load. NEFF executes once per load, so
    semaphores are not re-cleared at exit.
    """
    nc = bacc.Bacc(
        "TRN2",
        target_bir_lowering=False,
        debug=False,
        enable_asserts=False,
    )
    x = nc.dram_tensor("x", [E, S], F16, kind="ExternalInput").ap()
    vb_dram = nc.dram_tensor("vb", [P, S], F16, kind="ExternalInput").ap()
    out = nc.dram_tensor("out", [E, S], F16, kind="ExternalOutput").ap()

    n_tiles = E // P
    H = P // 2
    vb = nc.alloc_sbuf_tensor("vb_sb", [P, S], F16)
    xts = [
        nc.alloc_sbuf_tensor(f"xt{i}_sb", [P, S], F16) for i in range(n_tiles)
    ]
    ots = [
        nc.alloc_sbuf_tensor(f"ot{i}_sb", [P, S], F16) for i in range(n_tiles)
    ]
    t_sem = [nc.alloc_semaphore(f"t{i}_sem") for i in range(n_tiles)]
    v_sem = nc.alloc_semaphore("v_sem")
    v2_sem = nc.alloc_semaphore("v2_sem")
    s_sem = nc.alloc_semaphore("s_sem")
    st_sem = nc.alloc_semaphore("st_sem")

    _blk_cm = nc.Block(no_gpsimd_drain=True)
    blk = _blk_cm.__enter__()

    sp_loads = [0, 1, 4, 6]
    act_loads = [2, 3, 5, 7]

    @blk.sync
    def _(sync):
        # x0 and x1 on SP while vb streams on ACT: scan0 gates on exactly
        # two DMA completions (one per semaphore — completion updates to
        # one sem serialize at ~900ns each in the DMA update path), and
        # x1 is not queued behind the 512 KB vb transfer.
        for i in sp_loads:
            sync.dma_start(xts[i][:], x[i * P : (i + 1) * P, :]).then_inc(
                t_sem[i], 16
            )
        for i in range(0, n_tiles, 2):
            sync.wait_ge(s_sem, i + 1)
            sync.dma_start(
                out[i * P : (i + 1) * P, :], ots[i][:]
            ).then_inc(st_sem, 16)

    @blk.scalar
    def _(scalar):
        scalar.dma_start(vb[:H, :], vb_dram[:H, :]).then_inc(v_sem, 16)
        scalar.dma_start(vb[H:, :], vb_dram[H:, :]).then_inc(v2_sem, 16)
        for i in act_loads:
            scalar.dma_start(
                xts[i][:], x[i * P : (i + 1) * P, :]
            ).then_inc(t_sem[i], 16)
        for i in range(1, n_tiles, 2):
            scalar.wait_ge(s_sem, i + 1)
            scalar.dma_start(
                out[i * P : (i + 1) * P, :], ots[i][:]
            ).then_inc(st_sem, 16)

    @blk.vector
    def _(vector):
        vector.wait_ge(v_sem, 16)
        vector.wait_ge(v2_sem, 16)
        for i in range(n_tiles):
            vector.wait_ge(t_sem[i], 16)
            vector._custom_dve(
                _FUSED_OP, out=ots[i][:], in0=xts[i][:], in1=vb[:]
            ).then_inc(s_sem, 1)

    # Manual block exit: branch every engine to the end bb, but skip
    # Block.__exit__'s per-engine InstDrain (~4.3us DGE quiesce) and
    # all-engine barrier. Store completion is instead observed by SP
    # waiting for all 8 store-DMA semaphore increments, which keeps the
    # NEFF alive until the last output byte lands; engines with no wait
    # simply halt.
    for engine, last_body in blk.last_body.items():
        with nc.body(last_body, parent=nc.cur_bb, allow_existing_parent=True):
            engine.br(blk.end_bb)
    nc.switch_bb(blk.end_bb)
    nc.sync.wait_ge(st_sem, n_tiles * 16)
    # Sem-only all-engine barrier (no InstDrain): engines halt together,
    # so their end-of-NEFF flushes overlap instead of serializing.
    nc.all_engine_barrier(sem_only=True)
    nc.compile()
    return nc


def _build(d: float, has_bias: bool):
    """General path (any d in [0.9, 1], optional bias), fp32 throughout."""
    nc = bacc.Bacc(
        "TRN2",
        target_bir_lowering=False,
        debug=False,
        enable_asserts=False,
    )
    x = nc.dram_tensor("x", [E, S], F32, kind="ExternalInput").ap()
    vb_dram = nc.dram_tensor("vb", [P, S], F32, kind="ExternalInput").ap()
    bias_dram = None
    if has_bias:
        bias_dram = nc.dram_tensor("biasb", [P, S], F32, kind="ExternalInput").ap()
    out = nc.dram_tensor("out", [E, S], F32, kind="ExternalOutput").ap()

    with TileContext(nc) as tc:
        with (
            tc.tile_pool(name="const", bufs=1) as cpool,
            tc.tile_pool(name="xs", bufs=6) as xpool,
            tc.tile_pool(name="ys", bufs=2) as ypool,
            tc.tile_pool(name="os", bufs=4) as opool,
        ):
            # decay operand: [P, 1] column broadcast along the free axis
            dtile = cpool.tile([P, 1], F32)
            nc.gpsimd.memset(dtile[:], d)
            dbcast = dtile[:].broadcast_to([P, S])
            H = S // 2
            n_tiles = E // P
            vb = cpool.tile([P, S], F32)
            if has_bias:
                bb = cpool.tile([P, S], F32)
            rings = [nc.sync, nc.scalar, nc.gpsimd]
            rr = [0]

            def ring():
                r = rings[rr[0] % 3]
                rr[0] += 1
                return r

            for i in range(n_tiles):
                xt = xpool.tile([P, S], F32)
                ring().dma_start(out=xt[:], in_=x[i * P : (i + 1) * P, :])
                if i == 0:
                    nc.scalar.dma_start(out=vb[:], in_=vb_dram)
                    if has_bias:
                        nc.scalar.dma_start(out=bb[:], in_=bias_dram)
                yt = ypool.tile([P, S], F32)
                nc.vector.tensor_tensor_scan(
                    yt[:], dbcast, xt[:],
                    0.0, mybir.AluOpType.mult, mybir.AluOpType.add,
                )
                ot = opool.tile([P, S], F32)
                if i == n_tiles - 1:
                    # Last tile: split the mult so each half-store (on its
                    # own HWDGE ring) starts as soon as its half is ready.
                    nc.vector.tensor_mul(ot[:, :H], yt[:, :H], vb[:, :H])
                    if has_bias:
                        nc.vector.tensor_add(ot[:, :H], ot[:, :H], bb[:, :H])
                    nc.scalar.dma_start(
                        out=out[i * P : (i + 1) * P, :H], in_=ot[:, :H]
                    )
                    nc.vector.tensor_mul(ot[:, H:], yt[:, H:], vb[:, H:])
                    if has_bias:
                        nc.vector.tensor_add(ot[:, H:], ot[:, H:], bb[:, H:])
                    nc.sync.dma_start(
                        out=out[i * P : (i + 1) * P, H:], in_=ot[:, H:]
                    )
                else:
                    nc.vector.tensor_mul(ot[:], yt[:], vb[:])
                    if has_bias:
                        nc.vector.tensor_add(ot[:], ot[:], bb[:])
                    nc.scalar.dma_start(out=out[i * P : (i + 1) * P, :], in_=ot[:])
    nc.compile()
    return nc


def _run(x, weight, bias, decay_value, trace=False):
    x = np.asarray(x, dtype=np.float32)
    weight = np.asarray(weight, dtype=np.float32)
    bias = np.asarray(bias, dtype=np.float32)
    decay_value = np.asarray(decay_value)
    assert x.shape == (B, E, S), x.shape

    # DECAY_CONSTANT = 1.0 in the reference; exponent is (t - s) / 1.0.
    d = float(np.clip(np.float64(decay_value.reshape(-1)[0]), 0.9, 1.0))
    has_bias = bool(np.any(bias))

    if d == 1.0 and not has_bias and _FUSED_OP is not None:
        if "fp16" not in _cache:
            _cache["fp16"] = _build_fp16()
        nc = _cache["fp16"]
        vb = np.ascontiguousarray(
            np.broadcast_to(weight.reshape(1, S).astype(np.float16), (P, S))
        )
        in_maps = [
            {"x": x[b].astype(np.float16), "vb": vb} for b in range(N_CORES)
        ]
        res = run_bass_kernel_spmd(
            nc, in_maps, core_ids=list(range(N_CORES)), trace=trace
        )
        out = np.stack(
            [r["out"].astype(np.float32) for r in res.results], axis=0
        )
        return out, res

    key = (d, has_bias)
    if key not in _cache:
        _cache[key] = _build(d, has_bias)
    nc = _cache[key]

    vb = np.ascontiguousarray(
        np.broadcast_to(weight.reshape(1, S), (P, S)), dtype=np.float32
    )
    bb = None
    if has_bias:
        bb = np.ascontiguousarray(
            np.broadcast_to(bias.reshape(1, S), (P, S)), dtype=np.float32
        )

    in_maps = []
    for b in range(N_CORES):
        m = {"x": np.ascontiguousarray(x[b]), "vb": vb}
        if has_bias:
            m["biasb"] = bb
        in_maps.append(m)

    res = run_bass_kernel_spmd(
        nc, in_maps, core_ids=list(range(N_CORES)), trace=trace
    )
    out = np.stack([r["out"] for r in res.results], axis=0)
    return out, res


def kernel(x, weight, bias, decay_value):
    out, _ = _run(x, weight, bias, decay_value)
    return out


# revision 24
# speedup vs baseline: 1.1686x; 1.1686x over previous
"""Trainium2 Bass kernel for nn_ColRepeatCausalLinear.

Math: reference computes out = x @ W + bias with
    W[s, t] = v[t] * d^(t-s)  for t >= s, else 0,   d = clip(decay_value, 0.9, 1)
which factorizes as a decayed prefix scan along S:
    y[b, e, t] = d * y[b, e, t-1] + x[b, e, t]
    out[b, e, t] = v[t] * y[b, e, t] + bias[t]
i.e. O(B*E*S) work instead of the O(B*E*S^2) dense matmul.

Mapping: data-parallel over B across 8 NeuronCores (x[b] per core, params
replicated). Per core the kernel is DMA-bound (the scan+scale is one fused
Vector-engine op per 128x2048 tile), so I/O is done in fp16: the host casts
x/v to fp16 (quantization ~5e-4 L2 rel err, budget is 2e-2), the device
scans in fp32 internally (DVE ports upconvert), and the fp16 result is
upcast on the host. That halves HBM traffic: 8.4 MB/core instead of 16.8.
All 16+1 tiles live in SBUF simultaneously (68 KiB/partition of 208), so
every load issues at t=0 with no buffer-reuse (WAR) stalls, spread over the
three DGE rings (SP + ACT HWDGE, GpSimd SWDGE).

Hardcoded problem shapes: x (8, 1024, 2048) f32, weight (1, 2048),
bias (2048,), decay_value (1,).
"""

import numpy as np

import concourse.bacc as bacc
import concourse.mybir as mybir
from concourse.tile import TileContext
from concourse.bass_utils import run_bass_kernel_spmd

B, E, S = 8, 1024, 2048
P = 128
N_CORES = 8
F32 = mybir.dt.float32
F16 = mybir.dt.float16
I8 = mybir.dt.int8

_cache = {}

# Fused custom DVE op: out[p,k] = (sum_{j<=k} x[p,j]) * v[p,k] — the whole
# d=1 kernel body in ONE Vector-engine instruction (the stock path needs a
# 2-cyc/elem TensorTensorScan plus a 1-cyc/elem tensor_mul). Registered at
# runtime into dve_ops.OPS; sha self-pinned since this op isn't in-tree.
_FUSED_OP = None
try:
    from concourse import dve_ops as _dops
    from concourse.dve_spec import AluOp as _AluOp, Spec as _Spec
    from concourse.dve_spec import Src0 as _Src0, Src1 as _Src1, scan as _scan
    from concourse.dve_spec import lower as _lower
    from concourse.dve_uop import DveOpSpec as _DveOpSpec

    _FUSED_NAME = "CUMSUM_VSCALE_ANT"
    if _FUSED_NAME in _dops._SUB_OPCODE_FOR_NAME:
        _FUSED_OP = next(o for o in _dops.OPS if o.name == _FUSED_NAME)
    else:
        _fspec = _Spec(body=_scan(_AluOp.ADD, _Src0) * _Src1)
        _row = _dops._CUSTOM_DVE_ROW_BASE + len(_dops.OPS)
        assert _row < 0x20
        _dops._SUB_OPCODE_FOR_NAME[_FUSED_NAME] = _row
        _sha = {}
        for _ver in ("v3", "v4"):
            try:
                _sha[_ver] = _DveOpSpec(
                    name=_FUSED_NAME,
                    opcode=_row,
                    uops=_lower(_fspec, ver=_ver),
                    rd1_en=_dops.has_src1(_fspec),
                ).sha(_ver)
            except Exception:
                pass
        _FUSED_OP = _dops.DveOp(_FUSED_NAME, _fspec, subdim=False, uops_sha=_sha)
        _dops.OPS.append(_FUSED_OP)
        _dops.CUSTOM_DVE_SPECS[_FUSED_NAME] = _fspec
except Exception:
    _FUSED_OP = None


def _build_fp16():
    """Fast path: d == 1, no bias, fp16 I/O, fused scan*v DVE op.

    Raw bass (no TileContext): the dependency structure is static and
    tiny (17 DMAs, 8 scans), so explicit semaphores cost nothing and the
    TileContext exit epilogue (double all-engine barrier + event-sem
    RANGE_CLEAR + per-ring drains, ~8 us of a ~38 us kernel) collapses
    to one drain + sem-only barrier from Block exit.

    Schedule: loads split across the SP and ACT HWDGE rings in tile
    order (vb + tile0 split first so scan0 starts ~3 us in); the DVE
    chain of 8 fused scan*v ops is the latency backbone; stores issue
    from each ring as scans complete, queued after all loads so a
    blocked store never delays a load. NEFF executes once per load, so
    semaphores are not re-cleared at exit.
    """
    nc = bacc.Bacc(
        "TRN2",
        target_bir_lowering=False,
        debug=False,
        enable_asserts=False,
    )
    x = nc.dram_tensor("x", [E, S], F16, kind="ExternalInput").ap()
    vb_dram = nc.dram_tensor("vb", [P, S], F16, kind="ExternalInput").ap()
    out = nc.dram_tensor("out", [E, S], F16, kind="ExternalOutput").ap()

    n_tiles = E // P
    H = P // 2
    vb = nc.alloc_sbuf_tensor("vb_sb", [P, S], F16)
    xts = [
        nc.alloc_sbuf_tensor(f"xt{i}_sb", [P, S], F16) for i in range(n_tiles)
    ]
    ots = [
        nc.alloc_sbuf_tensor(f"ot{i}_sb", [P, S], F16) for i in range(n_tiles)
    ]
    t_sem = [nc.alloc_semaphore(f"t{i}_sem") for i in range(n_tiles)]
    v_sem = nc.alloc_semaphore("v_sem")
    s_sem = nc.alloc_semaphore("s_sem")
    st_sem = nc.alloc_semaphore("st_sem")

    _blk_cm = nc.Block(no_gpsimd_drain=True)
    blk = _blk_cm.__enter__()

    sp_loads = [0, 1, 4, 6]
    act_loads = [2, 3, 5, 7]

    @blk.sync
    def _(sync):
        # x0 and x1 on SP while vb streams on ACT: scan0 gates on exactly
        # two DMA completions (one per semaphore — completion updates to
        # one sem serialize at ~900ns each in the DMA update path), and
        # x1 is not queued behind the 512 KB vb transfer.
        for i in sp_loads:
            sync.dma_start(xts[i][:], x[i * P : (i + 1) * P, :]).then_inc(
                t_sem[i], 16
            )
        for i in range(0, n_tiles, 2):
            sync.wait_ge(s_sem, i + 1)
            sync.dma_start(
                out[i * P : (i + 1) * P, :], ots[i][:]
            ).then_inc(st_sem, 16)

    @blk.scalar
    def _(scalar):
        scalar.dma_start(vb[:], vb_dram[:]).then_inc(v_sem, 16)
        for i in act_loads:
            scalar.dma_start(
                xts[i][:], x[i * P : (i + 1) * P, :]
            ).then_inc(t_sem[i], 16)
        for i in range(1, n_tiles, 2):
            scalar.wait_ge(s_sem, i + 1)
            scalar.dma_start(
                out[i * P : (i + 1) * P, :], ots[i][:]
            ).then_inc(st_sem, 16)

    @blk.vector
    def _(vector):
        vector.wait_ge(v_sem, 16)
        for i in range(n_tiles):
            vector.wait_ge(t_sem[i], 16)
            vector._custom_dve(
                _FUSED_OP, out=ots[i][:], in0=xts[i][:], in1=vb[:]
            ).then_inc(s_sem, 1)

    # Manual block exit: branch every engine to the end bb, but skip
    # Block.__exit__'s per-engine InstDrain (~4.3us DGE quiesce) and
    # all-engine barrier. Store completion is instead observed by SP
    # waiting for all 8 store-DMA semaphore increments, which keeps the
    # NEFF alive until the last output byte lands; engines with no wait
    # simply halt.
    for engine, last_body in blk.last_body.items():
        with nc.body(last_body, parent=nc.cur_bb, allow_existing_parent=True):
            engine.br(blk.end_bb)
    nc.switch_bb(blk.end_bb)
    nc.sync.wait_ge(st_sem, n_tiles * 16)
    # Sem-only all-engine barrier (no InstDrain): engines halt together,
    # so their end-of-NEFF flushes overlap instead of serializing.
    nc.all_engine_barrier(sem_only=True)
    nc.compile()
    return nc


def _build(d: float, has_bias: bool):
    """General path (any d in [0.9, 1], optional bias), fp32 throughout."""
    nc = bacc.Bacc(
        "TRN2",
        target_bir_lowering=False,
        debug=False,
        enable_asserts=False,
    )
    x = nc.dram_tensor("x", [E, S], F32, kind="ExternalInput").ap()
    vb_dram = nc.dram_tensor("vb", [P, S], F32, kind="ExternalInput").ap()
    bias_dram = None
    if has_bias:
        bias_dram = nc.dram_tensor("biasb", [P, S], F32, kind="ExternalInput").ap()
    out = nc.dram_tensor("out", [E, S], F32, kind="ExternalOutput").ap()

    with TileContext(nc) as tc:
        with (
            tc.tile_pool(name="const", bufs=1) as cpool,
            tc.tile_pool(name="xs", bufs=6) as xpool,
            tc.tile_pool(name="ys", bufs=2) as ypool,
            tc.tile_pool(name="os", bufs=4) as opool,
        ):
            # decay operand: [P, 1] column broadcast along the free axis
            dtile = cpool.tile([P, 1], F32)
            nc.gpsimd.memset(dtile[:], d)
            dbcast = dtile[:].broadcast_to([P, S])
            H = S // 2
            n_tiles = E // P
            vb = cpool.tile([P, S], F32)
            if has_bias:
                bb = cpool.tile([P, S], F32)
            rings = [nc.sync, nc.scalar, nc.gpsimd]
            rr = [0]

            def ring():
                r = rings[rr[0] % 3]
                rr[0] += 1
                return r

            for i in range(n_tiles):
                xt = xpool.tile([P, S], F32)
                ring().dma_start(out=xt[:], in_=x[i * P : (i + 1) * P, :])
                if i == 0:
                    nc.scalar.dma_start(out=vb[:], in_=vb_dram)
                    if has_bias:
                        nc.scalar.dma_start(out=bb[:], in_=bias_dram)
                yt = ypool.tile([P, S], F32)
                nc.vector.tensor_tensor_scan(
                    yt[:], dbcast, xt[:],
                    0.0, mybir.AluOpType.mult, mybir.AluOpType.add,
                )
                ot = opool.tile([P, S], F32)
                if i == n_tiles - 1:
                    # Last tile: split the mult so each half-store (on its
                    # own HWDGE ring) starts as soon as its half is ready.
                    nc.vector.tensor_mul(ot[:, :H], yt[:, :H], vb[:, :H])
                    if has_bias:
                        nc.vector.tensor_add(ot[:, :H], ot[:, :H], bb[:, :H])
                    nc.scalar.dma_start(
                        out=out[i * P : (i + 1) * P, :H], in_=ot[:, :H]
                    )
                    nc.vector.tensor_mul(ot[:, H:], yt[:, H:], vb[:, H:])
                    if has_bias:
                        nc.vector.tensor_add(ot[:, H:], ot[:, H:], bb[:, H:])
                    nc.sync.dma_start(
                        out=out[i * P : (i + 1) * P, H:], in_=ot[:, H:]
                    )
                else:
                    nc.vector.tensor_mul(ot[:], yt[:], vb[:])
                    if has_bias:
                        nc.vector.tensor_add(ot[:], ot[:], bb[:])
                    nc.scalar.dma_start(out=out[i * P : (i + 1) * P, :], in_=ot[:])
    nc.compile()
    return nc


def _run(x, weight, bias, decay_value, trace=False):
    x = np.asarray(x, dtype=np.float32)
    weight = np.asarray(weight, dtype=np.float32)
    bias = np.asarray(bias, dtype=np.float32)
    decay_value = np.asarray(decay_value)
    assert x.shape == (B, E, S), x.shape

    # DECAY_CONSTANT = 1.0 in the reference; exponent is (t - s) / 1.0.
    d = float(np.clip(np.float64(decay_value.reshape(-1)[0]), 0.9, 1.0))
    has_bias = bool(np.any(bias))

    if d == 1.0 and not has_bias and _FUSED_OP is not None:
        if "fp16" not in _cache:
            _cache["fp16"] = _build_fp16()
        nc = _cache["fp16"]
        vb = np.ascontiguousarray(
            np.broadcast_to(weight.reshape(1, S).astype(np.float16), (P, S))
        )
        in_maps = [
            {"x": x[b].astype(np.float16), "vb": vb} for b in range(N_CORES)
        ]
        res = run_bass_kernel_spmd(
            nc, in_maps, core_ids=list(range(N_CORES)), trace=trace
        )
        out = np.stack(
            [r["out"].astype(np.float32) for r in res.results], axis=0
        )
        return out, res

    key = (d, has_bias)
    if key not in _cache:
        _cache[key] = _build(d, has_bias)
    nc = _cache[key]

    vb = np.ascontiguousarray(
        np.broadcast_to(weight.reshape(1, S), (P, S)), dtype=np.float32
    )
    bb = None
    if has_bias:
        bb = np.ascontiguousarray(
            np.broadcast_to(bias.reshape(1, S), (P, S)), dtype=np.float32
        )

    in_maps = []
    for b in range(N_CORES):
        m = {"x": np.ascontiguousarray(x[b]), "vb": vb}
        if has_bias:
            m["biasb"] = bb
        in_maps.append(m)

    res = run_bass_kernel_spmd(
        nc, in_maps, core_ids=list(range(N_CORES)), trace=trace
    )
    out = np.stack([r["out"] for r in res.results], axis=0)
    return out, res


def kernel(x, weight, bias, decay_value):
    out, _ = _run(x, weight, bias, decay_value)
    return out


# revision 25
# speedup vs baseline: 1.1890x; 1.0174x over previous
"""Trainium2 Bass kernel for nn_ColRepeatCausalLinear.

Math: reference computes out = x @ W + bias with
    W[s, t] = v[t] * d^(t-s)  for t >= s, else 0,   d = clip(decay_value, 0.9, 1)
which factorizes as a decayed prefix scan along S:
    y[b, e, t] = d * y[b, e, t-1] + x[b, e, t]
    out[b, e, t] = v[t] * y[b, e, t] + bias[t]
i.e. O(B*E*S) work instead of the O(B*E*S^2) dense matmul.

Mapping: data-parallel over B across 8 NeuronCores (x[b] per core, params
replicated). Per core the kernel is DMA-bound (the scan+scale is one fused
Vector-engine op per 128x2048 tile), so I/O is done in fp16: the host casts
x/v to fp16 (quantization ~5e-4 L2 rel err, budget is 2e-2), the device
scans in fp32 internally (DVE ports upconvert), and the fp16 result is
upcast on the host. That halves HBM traffic: 8.4 MB/core instead of 16.8.
All 16+1 tiles live in SBUF simultaneously (68 KiB/partition of 208), so
every load issues at t=0 with no buffer-reuse (WAR) stalls, spread over the
three DGE rings (SP + ACT HWDGE, GpSimd SWDGE).

Hardcoded problem shapes: x (8, 1024, 2048) f32, weight (1, 2048),
bias (2048,), decay_value (1,).
"""

import numpy as np

import concourse.bacc as bacc
import concourse.mybir as mybir
from concourse.tile import TileContext
from concourse.bass_utils import run_bass_kernel_spmd

B, E, S = 8, 1024, 2048
P = 128
N_CORES = 8
F32 = mybir.dt.float32
F16 = mybir.dt.float16
I8 = mybir.dt.int8

_cache = {}

# Fused custom DVE op: out[p,k] = (sum_{j<=k} x[p,j]) * v[p,k] — the whole
# d=1 kernel body in ONE Vector-engine instruction (the stock path needs a
# 2-cyc/elem TensorTensorScan plus a 1-cyc/elem tensor_mul). Registered at
# runtime into dve_ops.OPS; sha self-pinned since this op isn't in-tree.
_FUSED_OP = None
try:
    from concourse import dve_ops as _dops
    from concourse.dve_spec import AluOp as _AluOp, Spec as _Spec
    from concourse.dve_spec import Src0 as _Src0, Src1 as _Src1, scan as _scan
    from concourse.dve_spec import lower as _lower
    from concourse.dve_uop import DveOpSpec as _DveOpSpec

    _FUSED_NAME = "CUMSUM_VSCALE_ANT"
    if _FUSED_NAME in _dops._SUB_OPCODE_FOR_NAME:
        _FUSED_OP = next(o for o in _dops.OPS if o.name == _FUSED_NAME)
    else:
        _fspec = _Spec(body=_scan(_AluOp.ADD, _Src0) * _Src1)
        _row = _dops._CUSTOM_DVE_ROW_BASE + len(_dops.OPS)
        assert _row < 0x20
        _dops._SUB_OPCODE_FOR_NAME[_FUSED_NAME] = _row
        _sha = {}
        for _ver in ("v3", "v4"):
            try:
                _sha[_ver] = _DveOpSpec(
                    name=_FUSED_NAME,
                    opcode=_row,
                    uops=_lower(_fspec, ver=_ver),
                    rd1_en=_dops.has_src1(_fspec),
                ).sha(_ver)
            except Exception:
                pass
        _FUSED_OP = _dops.DveOp(_FUSED_NAME, _fspec, subdim=False, uops_sha=_sha)
        _dops.OPS.append(_FUSED_OP)
        _dops.CUSTOM_DVE_SPECS[_FUSED_NAME] = _fspec
except Exception:
    _FUSED_OP = None


def _build_fp16():
    """Fast path: d == 1, no bias, fp16 I/O, fused scan*v DVE op.

    Raw bass (no TileContext): the dependency structure is static and
    tiny (17 DMAs, 8 scans), so explicit semaphores cost nothing and the
    TileContext exit epilogue (double all-engine barrier + event-sem
    RANGE_CLEAR + per-ring drains, ~8 us of a ~38 us kernel) collapses
    to one drain + sem-only barrier from Block exit.

    Schedule: loads split across the SP and ACT HWDGE rings in tile
    order (vb + tile0 split first so scan0 starts ~3 us in); the DVE
    chain of 8 fused scan*v ops is the latency backbone; stores issue
    from each ring as scans complete, queued after all loads so a
    blocked store never delays a load. NEFF executes once per load, so
    semaphores are not re-cleared at exit.
    """
    nc = bacc.Bacc(
        "TRN2",
        target_bir_lowering=False,
        debug=False,
        enable_asserts=False,
    )
    x = nc.dram_tensor("x", [E, S], F16, kind="ExternalInput").ap()
    vb_dram = nc.dram_tensor("vb", [P, S], F16, kind="ExternalInput").ap()
    out = nc.dram_tensor("out", [E, S], F16, kind="ExternalOutput").ap()

    n_tiles = E // P
    H = P // 2
    vb = nc.alloc_sbuf_tensor("vb_sb", [P, S], F16)
    xts = [
        nc.alloc_sbuf_tensor(f"xt{i}_sb", [P, S], F16) for i in range(n_tiles)
    ]
    ots = [
        nc.alloc_sbuf_tensor(f"ot{i}_sb", [P, S], F16) for i in range(n_tiles)
    ]
    t_sem = [nc.alloc_semaphore(f"t{i}_sem") for i in range(n_tiles)]
    v_sem = nc.alloc_semaphore("v_sem")
    s_sem = nc.alloc_semaphore("s_sem")
    st_sem = nc.alloc_semaphore("st_sem")

    _blk_cm = nc.Block(no_gpsimd_drain=True)
    blk = _blk_cm.__enter__()

    sp_loads = [0, 1, 4, 6]
    act_loads = [2, 3, 5, 7]

    @blk.sync
    def _(sync):
        # x0 and x1 on SP while vb streams on ACT: scan0 gates on exactly
        # two DMA completions (one per semaphore — completion updates to
        # one sem serialize at ~900ns each in the DMA update path), and
        # x1 is not queued behind the 512 KB vb transfer.
        for i in sp_loads:
            sync.dma_start(xts[i][:], x[i * P : (i + 1) * P, :]).then_inc(
                t_sem[i], 16
            )
        for i in range(0, n_tiles, 2):
            sync.wait_ge(s_sem, i + 1)
            sync.dma_start(
                out[i * P : (i + 1) * P, :], ots[i][:]
            ).then_inc(st_sem, 16)
        # second half of the last tile's store (split with ACT so the
        # tail transfer halves)
        i = n_tiles - 1
        sync.wait_ge(s_sem, n_tiles)
        sync.dma_start(
            out[i * P + H :, :], ots[i][H:, :]
        ).then_inc(st_sem, 16)

    @blk.scalar
    def _(scalar):
        scalar.dma_start(vb[:], vb_dram[:]).then_inc(v_sem, 16)
        for i in act_loads:
            scalar.dma_start(
                xts[i][:], x[i * P : (i + 1) * P, :]
            ).then_inc(t_sem[i], 16)
        for i in range(1, n_tiles - 1, 2):
            scalar.wait_ge(s_sem, i + 1)
            scalar.dma_start(
                out[i * P : (i + 1) * P, :], ots[i][:]
            ).then_inc(st_sem, 16)
        i = n_tiles - 1
        scalar.wait_ge(s_sem, n_tiles)
        scalar.dma_start(
            out[i * P : i * P + H, :], ots[i][:H, :]
        ).then_inc(st_sem, 16)

    @blk.vector
    def _(vector):
        vector.wait_ge(v_sem, 16)
        for i in range(n_tiles):
            vector.wait_ge(t_sem[i], 16)
            vector._custom_dve(
                _FUSED_OP, out=ots[i][:], in0=xts[i][:], in1=vb[:]
            ).then_inc(s_sem, 1)

    # Manual block exit: branch every engine to the end bb, but skip
    # Block.__exit__'s per-engine InstDrain (~4.3us DGE quiesce) and
    # all-engine barrier. Store completion is instead observed by SP
    # waiting for all 8 store-DMA semaphore increments, which keeps the
    # NEFF alive until the last output byte lands; engines with no wait
    # simply halt.
    for engine, last_body in blk.last_body.items():
        with nc.body(last_body, parent=nc.cur_bb, allow_existing_parent=True):
            engine.br(blk.end_bb)
    nc.switch_bb(blk.end_bb)
    nc.sync.wait_ge(st_sem, (n_tiles + 1) * 16)
    # Sem-only all-engine barrier (no InstDrain): engines halt together,
    # so their end-of-NEFF flushes overlap instead of serializing.
    nc.all_engine_barrier(sem_only=True)
    nc.compile()
    return nc


def _build(d: float, has_bias: bool):
    """General path (any d in [0.9, 1], optional bias), fp32 throughout."""
    nc = bacc.Bacc(
        "TRN2",
        target_bir_lowering=False,
        debug=False,
        enable_asserts=False,
    )
    x = nc.dram_tensor("x", [E, S], F32, kind="ExternalInput").ap()
    vb_dram = nc.dram_tensor("vb", [P, S], F32, kind="ExternalInput").ap()
    bias_dram = None
    if has_bias:
        bias_dram = nc.dram_tensor("biasb", [P, S], F32, kind="ExternalInput").ap()
    out = nc.dram_tensor("out", [E, S], F32, kind="ExternalOutput").ap()

    with TileContext(nc) as tc:
        with (
            tc.tile_pool(name="const", bufs=1) as cpool,
            tc.tile_pool(name="xs", bufs=6) as xpool,
            tc.tile_pool(name="ys", bufs=2) as ypool,
            tc.tile_pool(name="os", bufs=4) as opool,
        ):
            # decay operand: [P, 1] column broadcast along the free axis
            dtile = cpool.tile([P, 1], F32)
            nc.gpsimd.memset(dtile[:], d)
            dbcast = dtile[:].broadcast_to([P, S])
            H = S // 2
            n_tiles = E // P
            vb = cpool.tile([P, S], F32)
            if has_bias:
                bb = cpool.tile([P, S], F32)
            rings = [nc.sync, nc.scalar, nc.gpsimd]
            rr = [0]

            def ring():
                r = rings[rr[0] % 3]
                rr[0] += 1
                return r

            for i in range(n_tiles):
                xt = xpool.tile([P, S], F32)
                ring().dma_start(out=xt[:], in_=x[i * P : (i + 1) * P, :])
                if i == 0:
                    nc.scalar.dma_start(out=vb[:], in_=vb_dram)
                    if has_bias:
                        nc.scalar.dma_start(out=bb[:], in_=bias_dram)
                yt = ypool.tile([P, S], F32)
                nc.vector.tensor_tensor_scan(
                    yt[:], dbcast, xt[:],
                    0.0, mybir.AluOpType.mult, mybir.AluOpType.add,
                )
                ot = opool.tile([P, S], F32)
                if i == n_tiles - 1:
                    # Last tile: split the mult so each half-store (on its
                    # own HWDGE ring) starts as soon as its half is ready.
                    nc.vector.tensor_mul(ot[:, :H], yt[:, :H], vb[:, :H])
                    if has_bias:
                        nc.vector.tensor_add(ot[:, :H], ot[:, :H], bb[:, :H])
                    nc.scalar.dma_start(
                        out=out[i * P : (i + 1) * P, :H], in_=ot[:, :H]
                    )
                    nc.vector.tensor_mul(ot[:, H:], yt[:, H:], vb[:, H:])
                    if has_bias:
                        nc.vector.tensor_add(ot[:, H:], ot[:, H:], bb[:, H:])
                    nc.sync.dma_start(
                        out=out[i * P : (i + 1) * P, H:], in_=ot[:, H:]
                    )
                else:
                    nc.vector.tensor_mul(ot[:], yt[:], vb[:])
                    if has_bias:
                        nc.vector.tensor_add(ot[:], ot[:], bb[:])
                    nc.scalar.dma_start(out=out[i * P : (i + 1) * P, :], in_=ot[:])
    nc.compile()
    return nc


def _run(x, weight, bias, decay_value, trace=False):
    x = np.asarray(x, dtype=np.float32)
    weight = np.asarray(weight, dtype=np.float32)
    bias = np.asarray(bias, dtype=np.float32)
    decay_value = np.asarray(decay_value)
    assert x.shape == (B, E, S), x.shape

    # DECAY_CONSTANT = 1.0 in the reference; exponent is (t - s) / 1.0.
    d = float(np.clip(np.float64(decay_value.reshape(-1)[0]), 0.9, 1.0))
    has_bias = bool(np.any(bias))

    if d == 1.0 and not has_bias and _FUSED_OP is not None:
        if "fp16" not in _cache:
            _cache["fp16"] = _build_fp16()
        nc = _cache["fp16"]
        vb = np.ascontiguousarray(
            np.broadcast_to(weight.reshape(1, S).astype(np.float16), (P, S))
        )
        in_maps = [
            {"x": x[b].astype(np.float16), "vb": vb} for b in range(N_CORES)
        ]
        res = run_bass_kernel_spmd(
            nc, in_maps, core_ids=list(range(N_CORES)), trace=trace
        )
        out = np.stack(
            [r["out"].astype(np.float32) for r in res.results], axis=0
        )
        return out, res

    key = (d, has_bias)
    if key not in _cache:
        _cache[key] = _build(d, has_bias)
    nc = _cache[key]

    vb = np.ascontiguousarray(
        np.broadcast_to(weight.reshape(1, S), (P, S)), dtype=np.float32
    )
    bb = None
    if has_bias:
        bb = np.ascontiguousarray(
            np.broadcast_to(bias.reshape(1, S), (P, S)), dtype=np.float32
        )

    in_maps = []
    for b in range(N_CORES):
        m = {"x": np.ascontiguousarray(x[b]), "vb": vb}
        if has_bias:
            m["biasb"] = bb
        in_maps.append(m)

    res = run_bass_kernel_spmd(
        nc, in_maps, core_ids=list(range(N_CORES)), trace=trace
    )
    out = np.stack([r["out"] for r in res.results], axis=0)
    return out, res


def kernel(x, weight, bias, decay_value):
    out, _ = _run(x, weight, bias, decay_value)
    return out


# revision 26
# speedup vs baseline: 1.1927x; 1.0032x over previous
"""Trainium2 Bass kernel for nn_ColRepeatCausalLinear.

Math: reference computes out = x @ W + bias with
    W[s, t] = v[t] * d^(t-s)  for t >= s, else 0,   d = clip(decay_value, 0.9, 1)
which factorizes as a decayed prefix scan along S:
    y[b, e, t] = d * y[b, e, t-1] + x[b, e, t]
    out[b, e, t] = v[t] * y[b, e, t] + bias[t]
i.e. O(B*E*S) work instead of the O(B*E*S^2) dense matmul.

Mapping: data-parallel over B across 8 NeuronCores (x[b] per core, params
replicated). Per core the kernel is DMA-bound (the scan+scale is one fused
Vector-engine op per 128x2048 tile), so I/O is done in fp16: the host casts
x/v to fp16 (quantization ~4e-4 L2 rel err, budget is 2e-2), the device
scans in fp32 internally (DVE ports upconvert), and the fp16 result is
upcast on the host. That halves HBM traffic: 8.4 MB/core instead of 16.8.
All 16+1 tiles live in SBUF simultaneously (68 KiB/partition of 208), so
every load issues immediately with no buffer-reuse (WAR) stalls, spread
over the two HWDGE rings (SP + ACT).

Hardcoded problem shapes: x (8, 1024, 2048) f32, weight (1, 2048),
bias (2048,), decay_value (1,).
"""

import numpy as np

import concourse.bacc as bacc
import concourse.mybir as mybir
from concourse.tile import TileContext
from concourse.bass_utils import run_bass_kernel_spmd

B, E, S = 8, 1024, 2048
P = 128
N_CORES = 8
F32 = mybir.dt.float32
F16 = mybir.dt.float16

_cache = {}

# Fused custom DVE op: out[p,k] = (sum_{j<=k} x[p,j]) * v[p,k] — the whole
# d=1 kernel body in ONE Vector-engine instruction (the stock path needs a
# 2-cyc/elem TensorTensorScan plus a 1-cyc/elem tensor_mul). Registered at
# runtime into dve_ops.OPS; sha self-pinned since this op isn't in-tree.
_FUSED_OP = None
try:
    from concourse import dve_ops as _dops
    from concourse.dve_spec import AluOp as _AluOp, Spec as _Spec
    from concourse.dve_spec import Src0 as _Src0, Src1 as _Src1, scan as _scan
    from concourse.dve_spec import lower as _lower
    from concourse.dve_uop import DveOpSpec as _DveOpSpec

    _FUSED_NAME = "CUMSUM_VSCALE_ANT"
    if _FUSED_NAME in _dops._SUB_OPCODE_FOR_NAME:
        _FUSED_OP = next(o for o in _dops.OPS if o.name == _FUSED_NAME)
    else:
        _fspec = _Spec(body=_scan(_AluOp.ADD, _Src0) * _Src1)
        _row = _dops._CUSTOM_DVE_ROW_BASE + len(_dops.OPS)
        assert _row < 0x20
        _dops._SUB_OPCODE_FOR_NAME[_FUSED_NAME] = _row
        _sha = {}
        for _ver in ("v3", "v4"):
            try:
                _sha[_ver] = _DveOpSpec(
                    name=_FUSED_NAME,
                    opcode=_row,
                    uops=_lower(_fspec, ver=_ver),
                    rd1_en=_dops.has_src1(_fspec),
                ).sha(_ver)
            except Exception:
                pass
        _FUSED_OP = _dops.DveOp(_FUSED_NAME, _fspec, subdim=False, uops_sha=_sha)
        _dops.OPS.append(_FUSED_OP)
        _dops.CUSTOM_DVE_SPECS[_FUSED_NAME] = _fspec
except Exception:
    _FUSED_OP = None


def _build_fp16():
    """Fast path: d == 1, no bias, fp16 I/O, fused scan*v DVE op.

    Raw bass (no TileContext): the dependency structure is static and
    tiny (18 DMAs, 8 scans), so explicit semaphores cost nothing and the
    TileContext exit epilogue (double all-engine barrier + event-sem
    RANGE_CLEAR + per-ring InstDrain, ~8 us of a ~38 us kernel)
    collapses to a store-completion semaphore wait plus one sem-only
    all-engine barrier (so all engines halt together and their
    end-of-NEFF flushes overlap).

    Schedule: loads split across the SP and ACT HWDGE rings in tile
    order (x0 on SP races vb on ACT, so scan0 gates on exactly two DMA
    completions — completion updates to one semaphore serialize at
    ~900ns each in the DMA update path, so gates use few, large DMAs);
    the DVE chain of 8 fused scan*v ops is the latency backbone; stores
    issue from each ring as scans complete, queued after all loads so a
    blocked store never delays a load; the last store is split across
    both rings to halve the tail transfer. NEFF executes once per load,
    so semaphores are not re-cleared at exit.
    """
    nc = bacc.Bacc(
        "TRN2",
        target_bir_lowering=False,
        debug=False,
        enable_asserts=False,
    )
    x = nc.dram_tensor("x", [E, S], F16, kind="ExternalInput").ap()
    vb_dram = nc.dram_tensor("vb", [P, S], F16, kind="ExternalInput").ap()
    out = nc.dram_tensor("out", [E, S], F16, kind="ExternalOutput").ap()

    n_tiles = E // P
    H = P // 2
    vb = nc.alloc_sbuf_tensor("vb_sb", [P, S], F16)
    xts = [
        nc.alloc_sbuf_tensor(f"xt{i}_sb", [P, S], F16) for i in range(n_tiles)
    ]
    ots = [
        nc.alloc_sbuf_tensor(f"ot{i}_sb", [P, S], F16) for i in range(n_tiles)
    ]
    t_sem = [nc.alloc_semaphore(f"t{i}_sem") for i in range(n_tiles)]
    v_sem = nc.alloc_semaphore("v_sem")
    s_sem = nc.alloc_semaphore("s_sem")
    st_sem = nc.alloc_semaphore("st_sem")

    _blk_cm = nc.Block(no_gpsimd_drain=True)
    blk = _blk_cm.__enter__()

    sp_loads = [0, 1, 4, 6]
    act_loads = [2, 3, 5, 7]

    @blk.sync
    def _(sync):
        # x0 and x1 on SP while vb streams on ACT: scan0 gates on exactly
        # two DMA completions (one per semaphore — completion updates to
        # one sem serialize at ~900ns each in the DMA update path), and
        # x1 is not queued behind the 512 KB vb transfer.
        for i in sp_loads:
            sync.dma_start(xts[i][:], x[i * P : (i + 1) * P, :]).then_inc(
                t_sem[i], 16
            )
        for i in range(0, n_tiles, 2):
            sync.wait_ge(s_sem, i + 1)
            sync.dma_start(
                out[i * P : (i + 1) * P, :], ots[i][:]
            ).then_inc(st_sem, 16)
        # second half of the last tile's store (split with ACT so the
        # tail transfer halves)
        i = n_tiles - 1
        sync.wait_ge(s_sem, n_tiles)
        sync.dma_start(
            out[i * P + H :, :], ots[i][H:, :]
        ).then_inc(st_sem, 16)

    @blk.scalar
    def _(scalar):
        scalar.dma_start(vb[:], vb_dram[:]).then_inc(v_sem, 16)
        for i in act_loads:
            scalar.dma_start(
                xts[i][:], x[i * P : (i + 1) * P, :]
            ).then_inc(t_sem[i], 16)
        for i in range(1, n_tiles - 1, 2):
            scalar.wait_ge(s_sem, i + 1)
            scalar.dma_start(
                out[i * P : (i + 1) * P, :], ots[i][:]
            ).then_inc(st_sem, 16)
        i = n_tiles - 1
        scalar.wait_ge(s_sem, n_tiles)
        scalar.dma_start(
            out[i * P : i * P + H, :], ots[i][:H, :]
        ).then_inc(st_sem, 16)

    @blk.vector
    def _(vector):
        vector.wait_ge(v_sem, 16)
        for i in range(n_tiles):
            vector.wait_ge(t_sem[i], 16)
            vector._custom_dve(
                _FUSED_OP, out=ots[i][:], in0=xts[i][:], in1=vb[:]
            ).then_inc(s_sem, 1)

    # Manual block exit: branch every engine to the end bb, but skip
    # Block.__exit__'s per-engine InstDrain (~4.3us DGE quiesce) and
    # all-engine barrier. Store completion is instead observed by SP
    # waiting for all 8 store-DMA semaphore increments, which keeps the
    # NEFF alive until the last output byte lands; engines with no wait
    # simply halt.
    for engine, last_body in blk.last_body.items():
        with nc.body(last_body, parent=nc.cur_bb, allow_existing_parent=True):
            engine.br(blk.end_bb)
    nc.switch_bb(blk.end_bb)
    nc.sync.wait_ge(st_sem, (n_tiles + 1) * 16)
    # Sem-only all-engine barrier (no InstDrain): engines halt together,
    # so their end-of-NEFF flushes overlap instead of serializing.
    nc.all_engine_barrier(sem_only=True)
    # Close the Block generator explicitly; BassBlock.__exit__ sees
    # GeneratorExit and emits nothing (the manual exit above replaced it).
    _blk_cm.gen.close()
    nc.cur_block = None
    nc.compile()
    return nc


def _build(d: float, has_bias: bool):
    """General path (any d in [0.9, 1], optional bias), fp32 throughout."""
    nc = bacc.Bacc(
        "TRN2",
        target_bir_lowering=False,
        debug=False,
        enable_asserts=False,
    )
    x = nc.dram_tensor("x", [E, S], F32, kind="ExternalInput").ap()
    vb_dram = nc.dram_tensor("vb", [P, S], F32, kind="ExternalInput").ap()
    bias_dram = None
    if has_bias:
        bias_dram = nc.dram_tensor("biasb", [P, S], F32, kind="ExternalInput").ap()
    out = nc.dram_tensor("out", [E, S], F32, kind="ExternalOutput").ap()

    with TileContext(nc) as tc:
        with (
            tc.tile_pool(name="const", bufs=1) as cpool,
            tc.tile_pool(name="xs", bufs=6) as xpool,
            tc.tile_pool(name="ys", bufs=2) as ypool,
            tc.tile_pool(name="os", bufs=4) as opool,
        ):
            # decay operand: [P, 1] column broadcast along the free axis
            dtile = cpool.tile([P, 1], F32)
            nc.gpsimd.memset(dtile[:], d)
            dbcast = dtile[:].broadcast_to([P, S])
            H = S // 2
            n_tiles = E // P
            vb = cpool.tile([P, S], F32)
            if has_bias:
                bb = cpool.tile([P, S], F32)
            rings = [nc.sync, nc.scalar, nc.gpsimd]
            rr = [0]

            def ring():
                r = rings[rr[0] % 3]
                rr[0] += 1
                return r

            for i in range(n_tiles):
                xt = xpool.tile([P, S], F32)
                ring().dma_start(out=xt[:], in_=x[i * P : (i + 1) * P, :])
                if i == 0:
                    nc.scalar.dma_start(out=vb[:], in_=vb_dram)
                    if has_bias:
                        nc.scalar.dma_start(out=bb[:], in_=bias_dram)
                yt = ypool.tile([P, S], F32)
                nc.vector.tensor_tensor_scan(
                    yt[:], dbcast, xt[:],
                    0.0, mybir.AluOpType.mult, mybir.AluOpType.add,
                )
                ot = opool.tile([P, S], F32)
                if i == n_tiles - 1:
                    # Last tile: split the mult so each half-store (on its
                    # own HWDGE ring) starts as soon as its half is ready.
                    nc.vector.tensor_mul(ot[:, :H], yt[:, :H], vb[:, :H])
                    if has_bias:
                        nc.vector.tensor_add(ot[:, :H], ot[:, :H], bb[:, :H])
                    nc.scalar.dma_start(
                        out=out[i * P : (i + 1) * P, :H], in_=ot[:, :H]
                    )
                    nc.vector.tensor_mul(ot[:, H:], yt[:, H:], vb[:, H:])
                    if has_bias:
                        nc.vector.tensor_add(ot[:, H:], ot[:, H:], bb[:, H:])
                    nc.sync.dma_start(
                        out=out[i * P : (i + 1) * P, H:], in_=ot[:, H:]
                    )
                else:
                    nc.vector.tensor_mul(ot[:], yt[:], vb[:])
                    if has_bias:
                        nc.vector.tensor_add(ot[:], ot[:], bb[:])
                    nc.scalar.dma_start(out=out[i * P : (i + 1) * P, :], in_=ot[:])
    nc.compile()
    return nc


def _run(x, weight, bias, decay_value, trace=False):
    x = np.asarray(x, dtype=np.float32)
    weight = np.asarray(weight, dtype=np.float32)
    bias = np.asarray(bias, dtype=np.float32)
    decay_value = np.asarray(decay_value)
    assert x.shape == (B, E, S), x.shape

    # DECAY_CONSTANT = 1.0 in the reference; exponent is (t - s) / 1.0.
    d = float(np.clip(np.float64(decay_value.reshape(-1)[0]), 0.9, 1.0))
    has_bias = bool(np.any(bias))

    if d == 1.0 and not has_bias and _FUSED_OP is not None:
        if "fp16" not in _cache:
            _cache["fp16"] = _build_fp16()
        nc = _cache["fp16"]
        vb = np.ascontiguousarray(
            np.broadcast_to(weight.reshape(1, S).astype(np.float16), (P, S))
        )
        in_maps = [
            {"x": x[b].astype(np.float16), "vb": vb} for b in range(N_CORES)
        ]
        res = run_bass_kernel_spmd(
            nc, in_maps, core_ids=list(range(N_CORES)), trace=trace
        )
        out = np.stack(
            [r["out"].astype(np.float32) for r in res.results], axis=0
        )
        return out, res

    key = (d, has_bias)
    if key not in _cache:
        _cache[key] = _build(d, has_bias)
    nc = _cache[key]

    vb = np.ascontiguousarray(
        np.broadcast_to(weight.reshape(1, S), (P, S)), dtype=np.float32
    )
    bb = None
    if has_bias:
        bb = np.ascontiguousarray(
            np.broadcast_to(bias.reshape(1, S), (P, S)), dtype=np.float32
        )

    in_maps = []
    for b in range(N_CORES):
        m = {"x": np.ascontiguousarray(x[b]), "vb": vb}
        if has_bias:
            m["biasb"] = bb
        in_maps.append(m)

    res = run_bass_kernel_spmd(
        nc, in_maps, core_ids=list(range(N_CORES)), trace=trace
    )
    out = np.stack([r["out"] for r in res.results], axis=0)
    return out, res


def kernel(x, weight, bias, decay_value):
    out, _ = _run(x, weight, bias, decay_value)
    return out
